# revision 16
# baseline (speedup 1.0000x reference)
"""3D Haar DWT (2x2x2 blocks, 8 subbands) on 8 Trainium2 NeuronCores.

Input  x: (2, 16, 64, 128, 128) f32.
Output: tuple of 8 subbands, each (2, 16, 32, 64, 64) f32, subband order
LLL,LLH,LHL,LHH,HLL,HLH,HHL,HHH (filters applied to (D,H,W) resp.).

Strategy (pure data parallel, zero cross-core communication), bf16 I/O:
  - The rel-err budget (2e-2) dwarfs bf16 quantization (~3e-3), so the
    host casts x to bf16 and the kernel reads/writes bf16 HBM — half the
    HBM traffic of f32, which is the binding roofline (memory regime):
    16 MiB per core -> ~47 us at the 358 GB/s per-core HBM limit.
  - Flatten (B,C) -> 32 independent slabs of (64,128,128); core i takes 4.
  - Per slab: partitions = (d, hh) [p = d*2+hh], free = (hb, q, w) -- each
    partition's free dim walks a CONTIGUOUS 4KB HBM region per quarter-DMA.
  - TensorE applies a constant 128x128 matrix on the partition axis for the
    D-axis butterfly (entries +/-0.5, exact in bf16).  The moving-tensor
    access pattern enumerates columns (q, r, hb, w2) so PSUM comes out with
    h-parity q and w-parity r DEINTERLEAVED into contiguous blocks.
  - ScalarE drains PSUM f32 -> SBUF bf16 with a fused *1/sqrt(2) scale
    (restoring the full 1/(2*sqrt2) Haar magnitude exactly).
  - The H (q) and W (r) butterflies run on the VectorEngine as pure
    add/sub over dense bf16 blocks -> 2x_1P packed mode throughout.
  - Each subband's slab result is one contiguous 256KB bf16 DMA to HBM.
"""

import numpy as np
import ml_dtypes

_B, _C, _D, _H, _W = 2, 16, 64, 128, 128
_NCORES = 8
_SLABS = _B * _C  # 32
_SLABS_PER_CORE = _SLABS // _NCORES  # 4
_BF16 = ml_dtypes.bfloat16
_SQRT_HALF = float(1.0 / np.sqrt(2.0))


def _haar_matrix():
    """(128,128) f32 (+/-0.5 entries; cast to bf16 exactly) for the D-axis
    butterfly on the partition axis.

    Input partition  = d*2 + hh          (hh = h-half, d = depth 0..63)
    Output partition = d'*4 + hh*2 + a   (a = D band, d' = 0..31)
    (d'-major order keeps the paired-subband store's DRAM-side AP outer
    dim at 32, so the HWDGE sprays its descriptors across all 16 SDMA
    engines -- an a-major order funnels every store into engines 0-1.)
    Entry = sign(f_a[p]) * 0.5  (d = 2d'+p).  The remaining 1/sqrt(2) of
    the Haar scale s^3 = 0.5/sqrt(2) is folded into the PSUM-drain copy on
    ScalarE, so the H/W butterflies on DVE are pure +/- adds and every
    matmul product is exact in f32 PSUM."""
    M = np.zeros((128, 128), dtype=np.float32)
    for hh in range(2):
        for a in range(2):
            for dp in range(32):
                for p in range(2):
                    sign = -1.0 if (a == 1 and p == 1) else 1.0
                    M[(2 * dp + p) * 2 + hh, dp * 4 + hh * 2 + a] = 0.5 * sign
    return M


def _build_bass():
    import concourse.mybir as mybir
    import concourse.tile as tile
    from concourse import bacc

    bf16 = mybir.dt.bfloat16
    f32 = mybir.dt.float32
    nc = bacc.Bacc("TRN2", target_bir_lowering=False, debug=False)

    x = nc.dram_tensor("x", [_SLABS_PER_CORE, _D, _H, _W], bf16, kind="ExternalInput")
    hm = nc.dram_tensor("hm", [128, 128], bf16, kind="ExternalInput")
    y = nc.dram_tensor(
        "y", [8, _SLABS_PER_CORE, _D // 2, _H // 2, _W // 2], bf16,
        kind="ExternalOutput",
    )

    # x[t, d, h, w] with h = hh*64 + hb*2 + q.
    # SBUF layout: partitions (d, hh) [p = d*2+hh], free (hb, q, w) -- each
    # partition's free dim walks a CONTIGUOUS HBM region (one descriptor
    # per partition), and the DRAM-side AP's outer dim is d:64, which the
    # HWDGE deals round-robin across all 16 SDMA engines.
    xr = x[:, :, :, :].rearrange("t d (hh hb q) w -> t d hh hb q w", hh=2, hb=32, q=2)
    # y[s, t, dp, h', w'] with h' = hh*32 + hb; s = a*4 + bg.  Grouping the
    # two D bands (a) of one (b,g) pair into a single DMA uses all 128
    # partitions of the o tile (a is the partition LSB after the matmul):
    # one 256KB 128-partition store per (b,g) per half-slab, with DRAM AP
    # order (dp, hh, a, hb, wp) matching partition order dp*4 + hh*2 + a.
    yr = y[:, :, :, :, :].rearrange(
        "(a bg) t dp (hh hb) wp -> bg t dp hh a hb wp", a=2, hh=2
    )

    with tile.TileContext(nc) as tc:
        with (
            tc.tile_pool(name="const", bufs=1) as cpool,
            tc.tile_pool(name="xin", bufs=8) as xpool,
            tc.tile_pool(name="uband", bufs=1) as upool,
            tc.tile_pool(name="outs", bufs=2) as opool,
            tc.tile_pool(name="stage", bufs=2) as spool,
            tc.tile_pool(name="psum", bufs=2, space="PSUM") as ppool,
        ):
            hmt = cpool.tile([128, 128], bf16, tag="hm")
            nc.sync.dma_start(out=hmt[:, :], in_=hm[:, :])

            def load_slab(t):
                # Whole slab: partitions (d, hh), free (hb, q, w) = 8192.
                # One 1MB DMA per half-slab (8KB contiguous per partition)
                # for near-peak SDMA efficiency.  Issue inputs via the
                # GPSIMD SWDGE queue: it is otherwise idle and has its own
                # ring, so input issue never queues behind output DMAs (SP
                # ring) or PSUM-drain copies (ACT), which caused PE stalls /
                # deadlocks on the HWDGE rings.  Slab 0's first half rides
                # the empty SP HWDGE ring instead (lower first-byte latency)
                # to cut kernel-start dead time.
                # Two half-slab tiles (bufs=4) so a prefetched load only
                # waits on the matmuls of the same half two slabs back.
                halves = []
                for h in range(2):
                    xh = xpool.tile([128, 4096], bf16, tag="xt", name=f"xt_{t}_{h}")
                    eng = nc.sync if (t == 0 and h == 0) else nc.gpsimd
                    eng.dma_start(
                        out=xh[:, :],
                        in_=xr[t, :, :, h * 16 : (h + 1) * 16],
                    )
                    halves.append(xh)
                return halves

            # Prefetch two slabs ahead (issued inside the loop): deep
            # enough that the input stream never starves compute, shallow
            # enough that the first chunks' loads don't time-share the SDMA
            # engines with many later loads (engines round-robin between
            # queues at packet granularity, so front-loading everything
            # makes the FIRST dependency finish as late as the last).
            xts = {0: load_slab(0), 1: load_slab(1)}
            for t in range(_SLABS_PER_CORE):
                xt = xts[t]
                if t + 2 < _SLABS_PER_CORE:
                    xts[t + 2] = load_slab(t + 2)

                # H-band intermediates (post D+H): free (c, r, hb, w2) = 4096.
                # Written and read only by DVE -> bufs=1 is race-free.
                u = [
                    upool.tile([128, 4096], bf16, tag=f"u{b}", name=f"u{b}_{t}")
                    for b in range(2)
                ]
                # Final subband tiles [beta][gamma]: free (hb, w') = 2048.
                o = [
                    [
                        opool.tile(
                            [128, 2048], bf16, tag=f"o{b}{g}", name=f"o{b}{g}_{t}"
                        )
                        for g in range(2)
                    ]
                    for b in range(2)
                ]

                last = t == _SLABS_PER_CORE - 1
                for half in range(2):
                    # Both chunks of the half drain into one ct tile so the
                    # H butterfly runs as two big N=2048 DVE ops (less
                    # fixed-cost per op; DVE is the longest compute pole).
                    ct = spool.tile([128, 4096], bf16, tag="ct")
                    for ci, c in enumerate((2 * half, 2 * half + 1)):
                        pt = ppool.tile([128, 2048], f32, tag="pt")
                        xth = xt[half]
                        # Moving-tensor AP walks (hb, w2) at fixed (q, r):
                        # PSUM free layout becomes (q, r, hb, w2) -- h-parity
                        # q and w-parity r land in contiguous 1024/512
                        # blocks, so the DVE butterflies below are dense
                        # 2x_1P bf16 ops.
                        xq = xth[:, :].rearrange(
                            "m (hb q w2 r) -> m q r hb w2", hb=16, q=2, w2=64, r=2
                        )
                        hb0 = (c % 2) * 8
                        for q in range(2):
                            for r in range(2):
                                j = q * 2 + r
                                nc.tensor.matmul(
                                    pt[:, j * 512 : (j + 1) * 512],
                                    hmt[:, :],
                                    xq[:, q, r, hb0 : hb0 + 8, :],
                                    start=True,
                                    stop=True,
                                )
                        # Drain PSUM f32 -> SBUF bf16 on ScalarE (otherwise
                        # idle), folding in the residual 1/sqrt(2) Haar
                        # scale.
                        nc.scalar.mul(
                            ct[:, ci * 2048 : (ci + 1) * 2048], pt[:, :], _SQRT_HALF
                        )
                    # H butterfly on DVE: q=0 block +/- q=1 block per chunk
                    # (2-segment APs, dense 1024-runs, 2x_1P).
                    cr = ct[:, :].rearrange("m (c q x) -> m c q x", c=2, q=2)
                    ev, od = cr[:, :, 0, :], cr[:, :, 1, :]
                    hs = slice(half * 2048, (half + 1) * 2048)
                    u0s = u[0][:, hs].rearrange("m (c x) -> m c x", c=2)
                    u1s = u[1][:, hs].rearrange("m (c x) -> m c x", c=2)
                    nc.vector.tensor_add(u0s, ev, od)
                    nc.vector.tensor_sub(u1s, ev, od)

                    # W butterfly per half-slab: r=0 block +/- r=1 block per
                    # chunk, dense 512-runs (2x_1P on DVE).  Half-slab
                    # granularity lets output DMAs start while the second
                    # half computes, halving the end-of-kernel store tail.
                    # The end-to-end time rides on the serial DVE stream
                    # (H+W = ~2.6us per chunk), so one of the four W ops is
                    # offloaded to GPSIMD, which is idle once input DMA
                    # issue is done.
                    os_ = slice(half * 1024, (half + 1) * 1024)
                    for b in range(2):
                        ur = u[b][:, hs].rearrange("m (c r x) -> m c r x", c=2, r=2)
                        ev, od = ur[:, :, 0, :], ur[:, :, 1, :]
                        o0 = o[b][0][:, os_].rearrange("m (c x) -> m c x", c=2)
                        o1 = o[b][1][:, os_].rearrange("m (c x) -> m c x", c=2)
                        nc.vector.tensor_add(o0, ev, od)
                        (nc.vector if b == 0 else nc.gpsimd).tensor_sub(o1, ev, od)

                    for bg in range(4):
                        b, g = bg >> 1, bg & 1
                        # One 128-partition store covers both D bands (a =
                        # partition MSB) of subband pair s = a*4 + bg; the
                        # half-slab covers o columns [half*1024, +1024) =
                        # hb in [16*half, +16).  Issue on SP so the ACT ring
                        # stays free to drain PSUM without delay; for the
                        # final slab (no more inputs/copies pending) spread
                        # across all rings to shrink the tail.
                        if last and half == 1:
                            eng = (nc.sync, nc.scalar, nc.gpsimd, nc.scalar)[bg]
                        else:
                            eng = nc.sync
                        eng.dma_start(
                            out=yr[bg, t, :, :, :, half * 16 : (half + 1) * 16, :],
                            in_=o[b][g][:, half * 1024 : (half + 1) * 1024],
                        )
    nc.compile()
    return nc


_NC_CACHE = None


def _get_nc():
    global _NC_CACHE
    if _NC_CACHE is None:
        _NC_CACHE = _build_bass()
    return _NC_CACHE


def _run(x, trace=False, **spmd_kwargs):
    from concourse.bass_utils import run_bass_kernel_spmd

    x = np.asarray(x, dtype=np.float32)
    xf = np.ascontiguousarray(x.reshape(_SLABS, _D, _H, _W)).astype(_BF16)
    M = _haar_matrix().astype(_BF16)
    in_maps = [
        {
            "x": np.ascontiguousarray(
                xf[i * _SLABS_PER_CORE : (i + 1) * _SLABS_PER_CORE]
            ),
            "hm": M,
        }
        for i in range(_NCORES)
    ]
    res = run_bass_kernel_spmd(
        _get_nc(), in_maps, core_ids=list(range(_NCORES)), trace=trace, **spmd_kwargs
    )
    outs = [r["y"] for r in res.results]  # each (8, 4, 32, 64, 64) bf16
    full = np.concatenate(outs, axis=1).astype(np.float32)  # (8, 32, 32, 64, 64)
    full = full.reshape(8, _B, _C, _D // 2, _H // 2, _W // 2)
    return full, res


def kernel(**inputs):
    full, _ = _run(inputs["x"])
    return tuple(full[i] for i in range(8))


# revision 22
# speedup vs baseline: 1.1907x; 1.1907x over previous
"""3D Haar DWT (2x2x2 blocks, 8 subbands) on 8 Trainium2 NeuronCores.

Input  x: (2, 16, 64, 128, 128) f32.
Output: tuple of 8 subbands, each (2, 16, 32, 64, 64) f32, subband order
LLL,LLH,LHL,LHH,HLL,HLH,HHL,HHH (filters applied to (D,H,W) resp.).

Strategy (pure data parallel, zero cross-core communication), bf16 I/O:
  - The rel-err budget (2e-2) dwarfs bf16 quantization (~3e-3), so the
    host casts x to bf16 and the kernel reads/writes bf16 HBM — half the
    HBM traffic of f32, which is the binding roofline (memory regime):
    16 MiB per core -> ~47 us at the 358 GB/s per-core HBM limit.
  - Flatten (B,C) -> 32 independent slabs of (64,128,128); core i takes 4.
  - Per slab: partitions = (d, hh) [p = d*2+hh], free = (hb, q, w) -- each
    partition's free dim walks a CONTIGUOUS 4KB HBM region per quarter-DMA.
  - TensorE applies a constant 128x128 matrix on the partition axis for the
    D-axis butterfly (entries +/-0.5, exact in bf16).  The moving-tensor
    access pattern enumerates columns (q, r, hb, w2) so PSUM comes out with
    h-parity q and w-parity r DEINTERLEAVED into contiguous blocks.
  - ScalarE drains PSUM f32 -> SBUF bf16 with a fused *1/sqrt(2) scale
    (restoring the full 1/(2*sqrt2) Haar magnitude exactly).
  - The H (q) and W (r) butterflies run on the VectorEngine as pure
    add/sub over dense bf16 blocks -> 2x_1P packed mode throughout.
  - Each subband's slab result is one contiguous 256KB bf16 DMA to HBM.
"""

import numpy as np
import ml_dtypes

_B, _C, _D, _H, _W = 2, 16, 64, 128, 128
_NCORES = 8
_SLABS = _B * _C  # 32
_SLABS_PER_CORE = _SLABS // _NCORES  # 4
_BF16 = ml_dtypes.bfloat16
_SQRT_HALF = float(1.0 / np.sqrt(2.0))


def _haar_matrix():
    """(128,128) f32 (+/-0.5 entries; cast to bf16 exactly) for the D-axis
    butterfly on the partition axis.

    Input partition  = d*2 + hh          (hh = h-half, d = depth 0..63)
    Output partition = d'*4 + hh*2 + a   (a = D band, d' = 0..31)
    (d'-major order keeps the paired-subband store's DRAM-side AP outer
    dim at 32, so the HWDGE sprays its descriptors across all 16 SDMA
    engines -- an a-major order funnels every store into engines 0-1.)
    Entry = sign(f_a[p]) * 0.5  (d = 2d'+p).  The remaining 1/sqrt(2) of
    the Haar scale s^3 = 0.5/sqrt(2) is folded into the PSUM-drain copy on
    ScalarE, so the H/W butterflies on DVE are pure +/- adds and every
    matmul product is exact in f32 PSUM."""
    M = np.zeros((128, 128), dtype=np.float32)
    for hh in range(2):
        for a in range(2):
            for dp in range(32):
                for p in range(2):
                    sign = -1.0 if (a == 1 and p == 1) else 1.0
                    M[(2 * dp + p) * 2 + hh, dp * 4 + hh * 2 + a] = 0.5 * sign
    return M


def _build_bass():
    import concourse.mybir as mybir
    import concourse.tile as tile
    from concourse import bacc

    bf16 = mybir.dt.bfloat16
    f32 = mybir.dt.float32
    nc = bacc.Bacc("TRN2", target_bir_lowering=False, debug=False)

    x = nc.dram_tensor("x", [_SLABS_PER_CORE, _D, _H, _W], bf16, kind="ExternalInput")
    hm = nc.dram_tensor("hm", [128, 128], bf16, kind="ExternalInput")
    y = nc.dram_tensor(
        "y", [8, _SLABS_PER_CORE, _D // 2, _H // 2, _W // 2], bf16,
        kind="ExternalOutput",
    )

    # x[t, d, h, w] with h = hh*64 + hb*2 + q.
    # SBUF layout: partitions (d, hh) [p = d*2+hh], free (hb, q, w) -- each
    # partition's free dim walks a CONTIGUOUS HBM region (one descriptor
    # per partition), and the DRAM-side AP's outer dim is d:64, which the
    # HWDGE deals round-robin across all 16 SDMA engines.
    xr = x[:, :, :, :].rearrange("t d (hh hb q) w -> t d hh hb q w", hh=2, hb=32, q=2)
    # y[s, t, dp, h', w'] with h' = hh*32 + hb; s = a*4 + bg.  Grouping the
    # two D bands (a) of one (b,g) pair into a single DMA uses all 128
    # partitions of the o tile (a is the partition LSB after the matmul):
    # one 256KB 128-partition store per (b,g) per half-slab, with DRAM AP
    # order (dp, hh, a, hb, wp) matching partition order dp*4 + hh*2 + a.
    yr = y[:, :, :, :, :].rearrange(
        "(a bg) t dp (hh hb) wp -> bg t dp hh a hb wp", a=2, hh=2
    )

    with tile.TileContext(nc) as tc:
        with (
            tc.tile_pool(name="const", bufs=1) as cpool,
            tc.tile_pool(name="xin", bufs=16) as xpool,
            tc.tile_pool(name="uband", bufs=1) as upool,
            tc.tile_pool(name="outs", bufs=2) as opool,
            tc.tile_pool(name="stage", bufs=2) as spool,
            tc.tile_pool(name="psum", bufs=2, space="PSUM") as ppool,
        ):
            hmt = cpool.tile([128, 128], bf16, tag="hm")
            nc.sync.dma_start(out=hmt[:, :], in_=hm[:, :])

            def load_slab(t):
                # Whole slab: partitions (d, hh), free (hb, q, w) = 8192.
                # One 512KB DMA per QUARTER-slab into its own tile (4KB
                # contiguous per partition): Tile dependencies are per-tile,
                # so each matmul chunk waits on exactly its own 512KB, not a
                # whole half/slab -- the compute stream starts ~5us earlier.
                # Issue inputs via the GPSIMD SWDGE queue: it is otherwise
                # idle and has its own ring, so input issue never queues
                # behind output DMAs (SP ring) or PSUM-drain copies (ACT),
                # which caused PE stalls / deadlocks on the HWDGE rings.
                # Slab 0's first two quarters ride the empty SP HWDGE ring
                # instead (lower first-byte latency) to cut start dead time.
                quarters = []
                for qt in range(4):
                    xq_t = xpool.tile([128, 2048], bf16, tag="xt", name=f"xt_{t}_{qt}")
                    eng = nc.sync if (t == 0 and qt < 2) else nc.gpsimd
                    eng.dma_start(
                        out=xq_t[:, :],
                        in_=xr[t, :, :, qt * 8 : (qt + 1) * 8],
                    )
                    quarters.append(xq_t)
                return quarters

            # Prefetch two slabs ahead (issued inside the loop): deep
            # enough that the input stream never starves compute, shallow
            # enough that the first chunks' loads don't time-share the SDMA
            # engines with many later loads (engines round-robin between
            # queues at packet granularity, so front-loading everything
            # makes the FIRST dependency finish as late as the last).
            xts = {0: load_slab(0), 1: load_slab(1)}
            for t in range(_SLABS_PER_CORE):
                xt = xts[t]
                if t + 2 < _SLABS_PER_CORE:
                    xts[t + 2] = load_slab(t + 2)

                # H-band intermediates (post D+H): free (c, r, hb, w2) = 4096.
                # Written and read only by DVE -> bufs=1 is race-free.
                u = [
                    upool.tile([128, 4096], bf16, tag=f"u{b}", name=f"u{b}_{t}")
                    for b in range(2)
                ]
                # Final subband tiles [beta][gamma]: free (hb, w') = 2048.
                o = [
                    [
                        opool.tile(
                            [128, 2048], bf16, tag=f"o{b}{g}", name=f"o{b}{g}_{t}"
                        )
                        for g in range(2)
                    ]
                    for b in range(2)
                ]

                last = t == _SLABS_PER_CORE - 1
                for half in range(2):
                    # Both chunks of the half drain into one ct tile so the
                    # H butterfly runs as two big N=2048 DVE ops (less
                    # fixed-cost per op; DVE is the longest compute pole).
                    ct = spool.tile([128, 4096], bf16, tag="ct")
                    for ci, c in enumerate((2 * half, 2 * half + 1)):
                        pt = ppool.tile([128, 2048], f32, tag="pt")
                        # Moving-tensor AP walks (hb, w2) at fixed (q, r):
                        # PSUM free layout becomes (q, r, hb, w2) -- h-parity
                        # q and w-parity r land in contiguous 1024/512
                        # blocks, so the DVE butterflies below are dense
                        # 2x_1P bf16 ops.
                        xq = xt[c][:, :].rearrange(
                            "m (hb q w2 r) -> m q r hb w2", hb=8, q=2, w2=64, r=2
                        )
                        for q in range(2):
                            for r in range(2):
                                j = q * 2 + r
                                nc.tensor.matmul(
                                    pt[:, j * 512 : (j + 1) * 512],
                                    hmt[:, :],
                                    xq[:, q, r, :, :],
                                    start=True,
                                    stop=True,
                                )
                        # Drain PSUM f32 -> SBUF bf16 on ScalarE (otherwise
                        # idle), folding in the residual 1/sqrt(2) Haar
                        # scale.
                        nc.scalar.mul(
                            ct[:, ci * 2048 : (ci + 1) * 2048], pt[:, :], _SQRT_HALF
                        )
                    # H butterfly on DVE: q=0 block +/- q=1 block per chunk
                    # (2-segment APs, dense 1024-runs, 2x_1P).
                    cr = ct[:, :].rearrange("m (c q x) -> m c q x", c=2, q=2)
                    ev, od = cr[:, :, 0, :], cr[:, :, 1, :]
                    hs = slice(half * 2048, (half + 1) * 2048)
                    u0s = u[0][:, hs].rearrange("m (c x) -> m c x", c=2)
                    u1s = u[1][:, hs].rearrange("m (c x) -> m c x", c=2)
                    nc.vector.tensor_add(u0s, ev, od)
                    nc.vector.tensor_sub(u1s, ev, od)

                    # W butterfly per half-slab: r=0 block +/- r=1 block per
                    # chunk, dense 512-runs (2x_1P on DVE).  Half-slab
                    # granularity lets output DMAs start while the second
                    # half computes, halving the end-of-kernel store tail.
                    # The end-to-end time rides on the serial DVE stream
                    # (H+W = ~2.6us per chunk), so one of the four W ops is
                    # offloaded to GPSIMD, which is idle once input DMA
                    # issue is done.
                    os_ = slice(half * 1024, (half + 1) * 1024)
                    for b in range(2):
                        ur = u[b][:, hs].rearrange("m (c r x) -> m c r x", c=2, r=2)
                        ev, od = ur[:, :, 0, :], ur[:, :, 1, :]
                        o0 = o[b][0][:, os_].rearrange("m (c x) -> m c x", c=2)
                        o1 = o[b][1][:, os_].rearrange("m (c x) -> m c x", c=2)
                        nc.vector.tensor_add(o0, ev, od)
                        nc.vector.tensor_sub(o1, ev, od)

                    for bg in range(4):
                        b, g = bg >> 1, bg & 1
                        # One 128-partition store covers both D bands (a =
                        # partition MSB) of subband pair s = a*4 + bg; the
                        # half-slab covers o columns [half*1024, +1024) =
                        # hb in [16*half, +16).  Issue on SP so the ACT ring
                        # stays free to drain PSUM without delay; for the
                        # final slab (no more inputs/copies pending) spread
                        # across all rings to shrink the tail.
                        if last and half == 1:
                            eng = (nc.sync, nc.scalar, nc.gpsimd, nc.scalar)[bg]
                        else:
                            eng = nc.sync
                        eng.dma_start(
                            out=yr[bg, t, :, :, :, half * 16 : (half + 1) * 16, :],
                            in_=o[b][g][:, half * 1024 : (half + 1) * 1024],
                        )
    nc.compile()
    return nc


_NC_CACHE = None


def _get_nc():
    global _NC_CACHE
    if _NC_CACHE is None:
        _NC_CACHE = _build_bass()
    return _NC_CACHE


def _run(x, trace=False, **spmd_kwargs):
    from concourse.bass_utils import run_bass_kernel_spmd

    x = np.asarray(x, dtype=np.float32)
    xf = np.ascontiguousarray(x.reshape(_SLABS, _D, _H, _W)).astype(_BF16)
    M = _haar_matrix().astype(_BF16)
    in_maps = [
        {
            "x": np.ascontiguousarray(
                xf[i * _SLABS_PER_CORE : (i + 1) * _SLABS_PER_CORE]
            ),
            "hm": M,
        }
        for i in range(_NCORES)
    ]
    res = run_bass_kernel_spmd(
        _get_nc(), in_maps, core_ids=list(range(_NCORES)), trace=trace, **spmd_kwargs
    )
    outs = [r["y"] for r in res.results]  # each (8, 4, 32, 64, 64) bf16
    full = np.concatenate(outs, axis=1).astype(np.float32)  # (8, 32, 32, 64, 64)
    full = full.reshape(8, _B, _C, _D // 2, _H // 2, _W // 2)
    return full, res


def kernel(**inputs):
    full, _ = _run(inputs["x"])
    return tuple(full[i] for i in range(8))


# revision 23
# speedup vs baseline: 1.2246x; 1.0285x over previous
"""3D Haar DWT (2x2x2 blocks, 8 subbands) on 8 Trainium2 NeuronCores.

Input  x: (2, 16, 64, 128, 128) f32.
Output: tuple of 8 subbands, each (2, 16, 32, 64, 64) f32, subband order
LLL,LLH,LHL,LHH,HLL,HLH,HHL,HHH (filters applied to (D,H,W) resp.).

Strategy (pure data parallel, zero cross-core communication), bf16 I/O:
  - The rel-err budget (2e-2) dwarfs bf16 quantization (~3e-3), so the
    host casts x to bf16 and the kernel reads/writes bf16 HBM — half the
    HBM traffic of f32, which is the binding roofline (memory regime):
    16 MiB per core -> ~47 us at the 358 GB/s per-core HBM limit.
  - Flatten (B,C) -> 32 independent slabs of (64,128,128); core i takes 4.
  - Per slab: partitions = (d, hh) [p = d*2+hh], free = (hb, q, w) -- each
    partition's free dim walks a CONTIGUOUS 4KB HBM region per quarter-DMA.
  - TensorE applies a constant 128x128 matrix on the partition axis for the
    D-axis butterfly (entries +/-0.5, exact in bf16).  The moving-tensor
    access pattern enumerates columns (q, r, hb, w2) so PSUM comes out with
    h-parity q and w-parity r DEINTERLEAVED into contiguous blocks.
  - ScalarE drains PSUM f32 -> SBUF bf16 with a fused *1/sqrt(2) scale
    (restoring the full 1/(2*sqrt2) Haar magnitude exactly).
  - The H (q) and W (r) butterflies run on the VectorEngine as pure
    add/sub over dense bf16 blocks -> 2x_1P packed mode throughout.
  - Each subband's slab result is one contiguous 256KB bf16 DMA to HBM.
"""

import numpy as np
import ml_dtypes

_B, _C, _D, _H, _W = 2, 16, 64, 128, 128
_NCORES = 8
_SLABS = _B * _C  # 32
_SLABS_PER_CORE = _SLABS // _NCORES  # 4
_BF16 = ml_dtypes.bfloat16
_SQRT_HALF = float(1.0 / np.sqrt(2.0))


def _haar_matrix():
    """(128,128) f32 (+/-0.5 entries; cast to bf16 exactly) for the D-axis
    butterfly on the partition axis.

    Input partition  = d*2 + hh          (hh = h-half, d = depth 0..63)
    Output partition = d'*4 + hh*2 + a   (a = D band, d' = 0..31)
    (d'-major order keeps the paired-subband store's DRAM-side AP outer
    dim at 32, so the HWDGE sprays its descriptors across all 16 SDMA
    engines -- an a-major order funnels every store into engines 0-1.)
    Entry = sign(f_a[p]) * 0.5  (d = 2d'+p).  The remaining 1/sqrt(2) of
    the Haar scale s^3 = 0.5/sqrt(2) is folded into the PSUM-drain copy on
    ScalarE, so the H/W butterflies on DVE are pure +/- adds and every
    matmul product is exact in f32 PSUM."""
    M = np.zeros((128, 128), dtype=np.float32)
    for hh in range(2):
        for a in range(2):
            for dp in range(32):
                for p in range(2):
                    sign = -1.0 if (a == 1 and p == 1) else 1.0
                    M[(2 * dp + p) * 2 + hh, dp * 4 + hh * 2 + a] = 0.5 * sign
    return M


def _build_bass():
    import concourse.mybir as mybir
    import concourse.tile as tile
    from concourse import bacc

    bf16 = mybir.dt.bfloat16
    f32 = mybir.dt.float32
    nc = bacc.Bacc("TRN2", target_bir_lowering=False, debug=False)

    x = nc.dram_tensor("x", [_SLABS_PER_CORE, _D, _H, _W], bf16, kind="ExternalInput")
    hm = nc.dram_tensor("hm", [128, 128], bf16, kind="ExternalInput")
    y = nc.dram_tensor(
        "y", [8, _SLABS_PER_CORE, _D // 2, _H // 2, _W // 2], bf16,
        kind="ExternalOutput",
    )

    # x[t, d, h, w] with h = hh*64 + hb*2 + q.
    # SBUF layout: partitions (d, hh) [p = d*2+hh], free (hb, q, w) -- each
    # partition's free dim walks a CONTIGUOUS HBM region (one descriptor
    # per partition), and the DRAM-side AP's outer dim is d:64, which the
    # HWDGE deals round-robin across all 16 SDMA engines.
    xr = x[:, :, :, :].rearrange("t d (hh hb q) w -> t d hh hb q w", hh=2, hb=32, q=2)
    # y[s, t, dp, h', w'] with h' = hh*32 + hb; s = a*4 + bg.  Grouping the
    # two D bands (a) of one (b,g) pair into a single DMA uses all 128
    # partitions of the o tile (a is the partition LSB after the matmul):
    # one 256KB 128-partition store per (b,g) per half-slab, with DRAM AP
    # order (dp, hh, a, hb, wp) matching partition order dp*4 + hh*2 + a.
    yr = y[:, :, :, :, :].rearrange(
        "(a bg) t dp (hh hb) wp -> bg t dp hh a hb wp", a=2, hh=2
    )

    with tile.TileContext(nc) as tc:
        with (
            tc.tile_pool(name="const", bufs=1) as cpool,
            tc.tile_pool(name="xin", bufs=16) as xpool,
            tc.tile_pool(name="uband", bufs=1) as upool,
            tc.tile_pool(name="outs", bufs=2) as opool,
            tc.tile_pool(name="stage", bufs=2) as spool,
            tc.tile_pool(name="psum", bufs=2, space="PSUM") as ppool,
        ):
            hmt = cpool.tile([128, 128], bf16, tag="hm")
            nc.sync.dma_start(out=hmt[:, :], in_=hm[:, :])

            def load_slab(t):
                # Whole slab: partitions (d, hh), free (hb, q, w) = 8192.
                # One 512KB DMA per QUARTER-slab into its own tile (4KB
                # contiguous per partition): Tile dependencies are per-tile,
                # so each matmul chunk waits on exactly its own 512KB, not a
                # whole half/slab -- the compute stream starts ~5us earlier.
                # Issue inputs via the GPSIMD SWDGE queue: it is otherwise
                # idle and has its own ring, so input issue never queues
                # behind output DMAs (SP ring) or PSUM-drain copies (ACT),
                # which caused PE stalls / deadlocks on the HWDGE rings.
                # Slab 0's first two quarters ride the empty SP HWDGE ring
                # instead (lower first-byte latency) to cut start dead time.
                quarters = []
                for qt in range(4):
                    xq_t = xpool.tile([128, 2048], bf16, tag="xt", name=f"xt_{t}_{qt}")
                    # Slab 0 loads split across BOTH rings (quarters 0/1 on
                    # the empty SP ring, 2/3 first in the GPSIMD queue) so
                    # its four chunks land in two parallel streams and the
                    # serial drain+butterfly pipeline starts ~3us earlier.
                    eng = nc.sync if (t == 0 and qt % 2 == 0) else nc.gpsimd
                    eng.dma_start(
                        out=xq_t[:, :],
                        in_=xr[t, :, :, qt * 8 : (qt + 1) * 8],
                    )
                    quarters.append(xq_t)
                return quarters

            # Prefetch two slabs ahead (issued inside the loop): deep
            # enough that the input stream never starves compute, shallow
            # enough that the first chunks' loads don't time-share the SDMA
            # engines with many later loads (engines round-robin between
            # queues at packet granularity, so front-loading everything
            # makes the FIRST dependency finish as late as the last).
            xts = {0: load_slab(0), 1: load_slab(1)}
            for t in range(_SLABS_PER_CORE):
                xt = xts[t]
                if t + 2 < _SLABS_PER_CORE:
                    xts[t + 2] = load_slab(t + 2)

                # H-band intermediates (post D+H): free (c, r, hb, w2) = 4096.
                # Written and read only by DVE -> bufs=1 is race-free.
                u = [
                    upool.tile([128, 4096], bf16, tag=f"u{b}", name=f"u{b}_{t}")
                    for b in range(2)
                ]
                # Final subband tiles [beta][gamma]: free (hb, w') = 2048.
                o = [
                    [
                        opool.tile(
                            [128, 2048], bf16, tag=f"o{b}{g}", name=f"o{b}{g}_{t}"
                        )
                        for g in range(2)
                    ]
                    for b in range(2)
                ]

                last = t == _SLABS_PER_CORE - 1
                for half in range(2):
                    # Both chunks of the half drain into one ct tile so the
                    # H butterfly runs as two big N=2048 DVE ops (less
                    # fixed-cost per op; DVE is the longest compute pole).
                    ct = spool.tile([128, 4096], bf16, tag="ct")
                    for ci, c in enumerate((2 * half, 2 * half + 1)):
                        pt = ppool.tile([128, 2048], f32, tag="pt")
                        # Moving-tensor AP walks (hb, w2) at fixed (q, r):
                        # PSUM free layout becomes (q, r, hb, w2) -- h-parity
                        # q and w-parity r land in contiguous 1024/512
                        # blocks, so the DVE butterflies below are dense
                        # 2x_1P bf16 ops.
                        xq = xt[c][:, :].rearrange(
                            "m (hb q w2 r) -> m q r hb w2", hb=8, q=2, w2=64, r=2
                        )
                        for q in range(2):
                            for r in range(2):
                                j = q * 2 + r
                                nc.tensor.matmul(
                                    pt[:, j * 512 : (j + 1) * 512],
                                    hmt[:, :],
                                    xq[:, q, r, :, :],
                                    start=True,
                                    stop=True,
                                )
                        # Drain PSUM f32 -> SBUF bf16 on ScalarE (otherwise
                        # idle), folding in the residual 1/sqrt(2) Haar
                        # scale.
                        nc.scalar.mul(
                            ct[:, ci * 2048 : (ci + 1) * 2048], pt[:, :], _SQRT_HALF
                        )
                    # H butterfly on DVE: q=0 block +/- q=1 block per chunk
                    # (2-segment APs, dense 1024-runs, 2x_1P).
                    cr = ct[:, :].rearrange("m (c q x) -> m c q x", c=2, q=2)
                    ev, od = cr[:, :, 0, :], cr[:, :, 1, :]
                    hs = slice(half * 2048, (half + 1) * 2048)
                    u0s = u[0][:, hs].rearrange("m (c x) -> m c x", c=2)
                    u1s = u[1][:, hs].rearrange("m (c x) -> m c x", c=2)
                    nc.vector.tensor_add(u0s, ev, od)
                    nc.vector.tensor_sub(u1s, ev, od)

                    # W butterfly per half-slab: r=0 block +/- r=1 block per
                    # chunk, dense 512-runs (2x_1P on DVE).  Half-slab
                    # granularity lets output DMAs start while the second
                    # half computes, halving the end-of-kernel store tail.
                    # The end-to-end time rides on the serial DVE stream
                    # (H+W = ~2.6us per chunk), so one of the four W ops is
                    # offloaded to GPSIMD, which is idle once input DMA
                    # issue is done.
                    os_ = slice(half * 1024, (half + 1) * 1024)
                    for b in range(2):
                        ur = u[b][:, hs].rearrange("m (c r x) -> m c r x", c=2, r=2)
                        ev, od = ur[:, :, 0, :], ur[:, :, 1, :]
                        o0 = o[b][0][:, os_].rearrange("m (c x) -> m c x", c=2)
                        o1 = o[b][1][:, os_].rearrange("m (c x) -> m c x", c=2)
                        nc.vector.tensor_add(o0, ev, od)
                        nc.vector.tensor_sub(o1, ev, od)

                    for bg in range(4):
                        b, g = bg >> 1, bg & 1
                        # One 128-partition store covers both D bands (a =
                        # partition MSB) of subband pair s = a*4 + bg; the
                        # half-slab covers o columns [half*1024, +1024) =
                        # hb in [16*half, +16).  Issue on SP so the ACT ring
                        # stays free to drain PSUM without delay; for the
                        # final slab (no more inputs/copies pending) spread
                        # across all rings to shrink the tail.
                        if last and half == 1:
                            eng = (nc.sync, nc.scalar, nc.gpsimd, nc.scalar)[bg]
                        else:
                            eng = nc.sync
                        eng.dma_start(
                            out=yr[bg, t, :, :, :, half * 16 : (half + 1) * 16, :],
                            in_=o[b][g][:, half * 1024 : (half + 1) * 1024],
                        )
    nc.compile()
    return nc


_NC_CACHE = None


def _get_nc():
    global _NC_CACHE
    if _NC_CACHE is None:
        _NC_CACHE = _build_bass()
    return _NC_CACHE


def _run(x, trace=False, **spmd_kwargs):
    from concourse.bass_utils import run_bass_kernel_spmd

    x = np.asarray(x, dtype=np.float32)
    xf = np.ascontiguousarray(x.reshape(_SLABS, _D, _H, _W)).astype(_BF16)
    M = _haar_matrix().astype(_BF16)
    in_maps = [
        {
            "x": np.ascontiguousarray(
                xf[i * _SLABS_PER_CORE : (i + 1) * _SLABS_PER_CORE]
            ),
            "hm": M,
        }
        for i in range(_NCORES)
    ]
    res = run_bass_kernel_spmd(
        _get_nc(), in_maps, core_ids=list(range(_NCORES)), trace=trace, **spmd_kwargs
    )
    outs = [r["y"] for r in res.results]  # each (8, 4, 32, 64, 64) bf16
    full = np.concatenate(outs, axis=1).astype(np.float32)  # (8, 32, 32, 64, 64)
    full = full.reshape(8, _B, _C, _D // 2, _H // 2, _W // 2)
    return full, res


def kernel(**inputs):
    full, _ = _run(inputs["x"])
    return tuple(full[i] for i in range(8))
